# revision 8
# baseline (speedup 1.0000x reference)
"""Trainium2 Bass kernel for nn_LuongAttnDecoderMogLSTM.

Strategy (8 NeuronCores on one chip):
  - MogLSTM / fc / concat matmuls: weight-sharded over the output dim
    (1/8 slice per core), activations kept transposed [feature, batch];
    slices AllGathered after each step (gather = concat on first dim).
  - Layer-2 MogLSTM has zero-init state, so its mogrifier collapses to a
    constant per-feature scale (8*sig(b0)*sig(b2)*sig(b4)) and lstm2_Whh /
    the f-gate are never needed.
  - Luong attention: batch-sharded (16 rows per core), single pass over
    encoder_outputs with online softmax; scores on DVE (fused
    multiply+accumulate), context via block-diagonal PE matmuls.
  - Output projection: vocab-sharded (4000 cols per core); softmax uses a
    single AllReduce of the per-core exp-sums (logits are tanh-bounded so
    no max subtraction is needed).
"""
import sys

for _p in ("/opt/trn_rl_repo", "/root/.axon_site/_ro/trn_rl_repo"):
    if _p not in sys.path:
        sys.path.insert(0, _p)

import numpy as np

import concourse.bass as bass
import concourse.mybir as mybir
import concourse.tile as tile
from concourse import bacc
from concourse.bass_utils import run_bass_kernel_spmd
from concourse.masks import make_identity

F32 = mybir.dt.float32
AF = mybir.ActivationFunctionType
OP = mybir.AluOpType
AX = mybir.AxisListType

NC = 8          # cores
B = 128         # batch
H = 1024        # hidden
S = 512         # encoder length
V = 32000       # vocab
P = 128         # partitions
KC = H // P     # 1024 = 8 k-chunks of 128
HS = H // NC    # 128   per-core slice of H-sized outputs
GQ = 4          # lstm1 quadrants (i,f,g,o)
G2Q = 3         # lstm2 quadrants kept (i,g,o)
BS = B // NC    # 16    batch shard for attention
SC = 4          # s-chunks of 128
VS = V // NC    # 4000  vocab slice
VT = 8          # vocab n-chunks of 500
VN = VS // VT   # 500

RG = [list(range(NC))]


def _ag_pair(nc, name, in_shape, dtype=F32):
    bin_ = nc.dram_tensor(f"{name}_agin", list(in_shape), dtype)
    out_shape = [in_shape[0] * NC] + list(in_shape[1:])
    bout = nc.dram_tensor(f"{name}_agout", out_shape, dtype, addr_space="Shared")
    return bin_, bout


def build_program():
    nc = bacc.Bacc("TRN2", target_bir_lowering=False, debug=False, num_devices=NC)

    def inp(name, shape):
        return nc.dram_tensor(name, list(shape), F32, kind="ExternalInput")

    x0T = inp("x0T", [H, B])
    h1T = inp("h1T", [H, B])
    c1T = inp("c1T", [H, B])
    mog1w = inp("mog1w", [5, H, HS])      # W_r.T column slice
    mog1b = inp("mog1b", [P, 5])
    wih1 = inp("wih1", [H, GQ * P])       # quadrant-interleaved col slice
    whh1 = inp("whh1", [H, GQ * P])
    b1 = inp("b1", [P, GQ])
    wih2 = inp("wih2", [H, G2Q * P])
    b2 = inp("b2", [P, G2Q])
    scale2 = inp("scale2", [P, KC])
    fcw = inp("fcw", [H, HS])
    fcb = inp("fcb", [P, 1])
    cw = inp("cw", [2 * H, HS])
    cb = inp("cb", [P, 1])
    encB = inp("encB", [S, BS, H])
    owT = inp("owT", [H, VS])
    ob = inp("ob", [1, VS])

    out_probs = nc.dram_tensor("out_probs", [B, VS], F32, kind="ExternalOutput")
    h2T_out = nc.dram_tensor("h2T_out", [H, B], F32, kind="ExternalOutput")

    # collective bounce buffers
    mog_ag = [_ag_pair(nc, f"mog{r}", [HS, B]) for r in range(5)]
    g1_ag = _ag_pair(nc, "g1", [GQ * P, B])
    g2_ag = _ag_pair(nc, "g2", [G2Q * P, B])
    fc_ag = _ag_pair(nc, "fc", [HS, B])
    ctx_ag = _ag_pair(nc, "ctx", [BS, H])
    cat_ag = _ag_pair(nc, "cat", [HS, B])
    sum_in = nc.dram_tensor("sum_agin", [B, 1], F32)
    sum_out = nc.dram_tensor("sum_agout", [B, 1], F32, addr_space="Shared")

    rnn_nat_dram = nc.dram_tensor("rnn_nat_dram", [B, H], F32)
    myrnn_dram = nc.dram_tensor("myrnn_dram", [BS, H], F32)
    mx_dram = [nc.dram_tensor(f"mx_dram{c}", [BS, 1], F32) for c in range(SC)]

    def t_layout(ap):  # [H, X] dram -> [128, KC, X] (h = 128*chunk + p)
        return ap.rearrange("(o p) b -> p o b", p=P)

    with tile.TileContext(nc) as tc:
        with (
            tc.tile_pool(name="pp", bufs=1) as pp,
            tc.tile_pool(name="psA", bufs=2, space="PSUM") as psA,
            tc.tile_pool(name="psT", bufs=1, space="PSUM") as psT,
        ):
            ident = pp.tile([P, P], F32, tag="ident")
            make_identity(nc, ident[:])
            ones = pp.tile([P, 1], F32, tag="ones")
            nc.vector.memset(ones[:], 1.0)

            # persistent activations (cross-phase)
            rnnT = pp.tile([P, KC, B], F32, tag="rnnT")
            ctxT = pp.tile([P, KC, B], F32, tag="ctxT")
            catT = pp.tile([P, KC, B], F32, tag="catT")

            # small constants
            m1b = pp.tile([P, 5], F32, tag="m1b")
            nc.sync.dma_start(m1b[:], mog1b.ap())
            b1t = pp.tile([P, GQ], F32, tag="b1t")
            nc.sync.dma_start(b1t[:], b1.ap())
            b2t = pp.tile([P, G2Q], F32, tag="b2t")
            nc.sync.dma_start(b2t[:], b2.ap())
            sc2 = pp.tile([P, KC], F32, tag="sc2")
            nc.sync.dma_start(sc2[:], scale2.ap())
            fcbt = pp.tile([P, 1], F32, tag="fcbt")
            nc.sync.dma_start(fcbt[:], fcb.ap())
            cbt = pp.tile([P, 1], F32, tag="cbt")
            nc.sync.dma_start(cbt[:], cb.ap())

            with tc.tile_pool(name="encp", bufs=2) as encp:
                # ---------------- phase A: MogLSTM + fc (weight-sharded) -------------
                with (
                    tc.tile_pool(name="pa", bufs=1) as pa,
                    tc.tile_pool(name="paw", bufs=3) as paw,
                    tc.tile_pool(name="pax", bufs=2) as pax,
                ):
                    xt = pax.tile([P, KC, B], F32, tag="xt")
                    nc.sync.dma_start(xt[:], t_layout(x0T.ap()))
                    ht = pax.tile([P, KC, B], F32, tag="ht")
                    nc.sync.dma_start(ht[:], t_layout(h1T.ap()))
                    ct = pa.tile([P, KC, B], F32, tag="ct")
                    nc.sync.dma_start(ct[:], t_layout(c1T.ap()))

                    # 5 mogrifier rounds
                    for r in range(5):
                        wt = paw.tile([P, KC, HS], F32, tag="mogw")
                        nc.sync.dma_start(wt[:], t_layout(mog1w.ap()[r]))
                        src = ht if r % 2 == 0 else xt
                        ps = psA.tile([HS, B], F32, tag="g")
                        for k in range(KC):
                            nc.tensor.matmul(ps[:], wt[:, k, :], src[:, k, :],
                                             start=(k == 0), stop=(k == KC - 1))
                        gs = pa.tile([HS, B], F32, tag="gs")
                        nc.scalar.activation(gs[:], ps[:], AF.Sigmoid,
                                             bias=m1b[:, r:r + 1])
                        bin_, bout = mog_ag[r]
                        nc.sync.dma_start(bin_.ap(), gs[:])
                        nc.gpsimd.collective_compute(
                            "AllGather", OP.bypass, replica_groups=RG,
                            ins=[bin_.ap().opt()], outs=[bout.ap().opt()])
                        gfull = pa.tile([P, KC, B], F32, tag="gfull")
                        nc.sync.dma_start(gfull[:], t_layout(bout.ap()))
                        if r % 2 == 0:  # update x
                            xn = pax.tile([P, KC, B], F32, tag="xt")
                            nc.vector.scalar_tensor_tensor(
                                out=xn[:], in0=gfull[:], scalar=2.0, in1=xt[:],
                                op0=OP.mult, op1=OP.mult)
                            xt = xn
                        else:
                            hn = pax.tile([P, KC, B], F32, tag="ht")
                            nc.vector.scalar_tensor_tensor(
                                out=hn[:], in0=gfull[:], scalar=2.0, in1=ht[:],
                                op0=OP.mult, op1=OP.mult)
                            ht = hn

                    # lstm1 gates (4 quadrant chunks, sliced by core)
                    gsl = pa.tile([P, GQ, B], F32, tag="gsl")
                    for q in range(GQ):
                        wi = paw.tile([P, KC, P], F32, tag="wq")
                        nc.sync.dma_start(
                            wi[:], t_layout(wih1.ap()[:, q * P:(q + 1) * P]))
                        wh = paw.tile([P, KC, P], F32, tag="wq")
                        nc.sync.dma_start(
                            wh[:], t_layout(whh1.ap()[:, q * P:(q + 1) * P]))
                        ps = psA.tile([P, B], F32, tag="g")
                        for k in range(KC):
                            nc.tensor.matmul(ps[:], wi[:, k, :], xt[:, k, :],
                                             start=(k == 0), stop=False)
                        for k in range(KC):
                            nc.tensor.matmul(ps[:], wh[:, k, :], ht[:, k, :],
                                             start=False, stop=(k == KC - 1))
                        fn = AF.Tanh if q == 2 else AF.Sigmoid
                        nc.scalar.activation(gsl[:, q, :], ps[:], fn,
                                             bias=b1t[:, q:q + 1])
                    bin_, bout = g1_ag
                    nc.sync.dma_start(
                        bin_.ap().rearrange("(q p) b -> p q b", p=P), gsl[:])
                    nc.gpsimd.collective_compute(
                        "AllGather", OP.bypass, replica_groups=RG,
                        ins=[bin_.ap().opt()], outs=[bout.ap().opt()])
                    g1v = bout.ap().rearrange("(r q p) b -> q p r b", q=GQ, p=P)
                    quad = []
                    for q in range(GQ):
                        qt = pa.tile([P, KC, B], F32, tag=f"quad{q}")
                        nc.sync.dma_start(qt[:], g1v[q])
                        quad.append(qt)
                    si, sf, tg, so = quad

                    # cell 1 (transposed layout); temp tiles chain through
                    # three reused tags to bound SBUF
                    t1 = pa.tile([P, KC, B], F32, tag="tmpA")
                    nc.vector.tensor_tensor(t1[:], si[:], tg[:], OP.mult)
                    t2 = pa.tile([P, KC, B], F32, tag="tmpB")
                    nc.vector.tensor_tensor(t2[:], sf[:], ct[:], OP.mult)
                    c1n = pa.tile([P, KC, B], F32, tag="tmpC")
                    nc.vector.tensor_tensor(c1n[:], t1[:], t2[:], OP.add)
                    tc1 = pa.tile([P, KC, B], F32, tag="tmpA")
                    nc.scalar.activation(tc1[:], c1n[:], AF.Tanh)
                    h1n = pa.tile([P, KC, B], F32, tag="tmpB")
                    nc.vector.tensor_tensor(h1n[:], so[:], tc1[:], OP.mult)

                    # layer 2: x2 = h1n * scale2 (zero-state mogrifier folded)
                    x2 = pa.tile([P, KC, B], F32, tag="tmpC")
                    for j in range(KC):
                        nc.vector.tensor_scalar_mul(
                            out=x2[:, j, :], in0=h1n[:, j, :],
                            scalar1=sc2[:, j:j + 1])
                    g2sl = pa.tile([P, G2Q, B], F32, tag="g2sl")
                    for q in range(G2Q):
                        wi = paw.tile([P, KC, P], F32, tag="wq")
                        nc.sync.dma_start(
                            wi[:], t_layout(wih2.ap()[:, q * P:(q + 1) * P]))
                        ps = psA.tile([P, B], F32, tag="g")
                        for k in range(KC):
                            nc.tensor.matmul(ps[:], wi[:, k, :], x2[:, k, :],
                                             start=(k == 0), stop=(k == KC - 1))
                        fn = AF.Tanh if q == 1 else AF.Sigmoid
                        nc.scalar.activation(g2sl[:, q, :], ps[:], fn,
                                             bias=b2t[:, q:q + 1])
                    bin_, bout = g2_ag
                    nc.sync.dma_start(
                        bin_.ap().rearrange("(q p) b -> p q b", p=P), g2sl[:])
                    nc.gpsimd.collective_compute(
                        "AllGather", OP.bypass, replica_groups=RG,
                        ins=[bin_.ap().opt()], outs=[bout.ap().opt()])
                    g2v = bout.ap().rearrange("(r q p) b -> q p r b", q=G2Q, p=P)
                    q2 = []
                    for q in range(G2Q):
                        qt = pa.tile([P, KC, B], F32, tag=f"quad{q}")
                        nc.sync.dma_start(qt[:], g2v[q])
                        q2.append(qt)
                    si2, tg2, so2 = q2
                    c2n = pa.tile([P, KC, B], F32, tag="tmpA")
                    nc.vector.tensor_tensor(c2n[:], si2[:], tg2[:], OP.mult)
                    tc2 = pa.tile([P, KC, B], F32, tag="tmpB")
                    nc.scalar.activation(tc2[:], c2n[:], AF.Tanh)
                    h2 = pa.tile([P, KC, B], F32, tag="h2")
                    nc.vector.tensor_tensor(h2[:], so2[:], tc2[:], OP.mult)
                    nc.sync.dma_start(t_layout(h2T_out.ap()), h2[:])

                    # fc -> rnn_out (transposed, gathered)
                    fw = paw.tile([P, KC, HS], F32, tag="mogw")
                    nc.sync.dma_start(fw[:], t_layout(fcw.ap()))
                    ps = psA.tile([HS, B], F32, tag="g")
                    for k in range(KC):
                        nc.tensor.matmul(ps[:], fw[:, k, :], h2[:, k, :],
                                         start=(k == 0), stop=(k == KC - 1))
                    rsl = pa.tile([HS, B], F32, tag="rsl")
                    nc.scalar.activation(rsl[:], ps[:], AF.Identity, bias=fcbt[:])
                    bin_, bout = fc_ag
                    nc.sync.dma_start(bin_.ap(), rsl[:])
                    nc.gpsimd.collective_compute(
                        "AllGather", OP.bypass, replica_groups=RG,
                        ins=[bin_.ap().opt()], outs=[bout.ap().opt()])
                    nc.sync.dma_start(rnnT[:], t_layout(bout.ap()))

                    # rnn_out natural [B, H] -> dram (for the per-core slice bcast)
                    rnat = pa.tile([P, H], F32, tag="rnat")
                    for j in range(KC):
                        pt = psT.tile([P, P], F32, tag="tr")
                        nc.tensor.transpose(pt[:], rnnT[:, j, :], ident[:])
                        nc.scalar.copy(rnat[:, j * P:(j + 1) * P], pt[:])
                    nc.sync.dma_start(rnn_nat_dram.ap(), rnat[:])

                # my 16 batch rows of rnn_out (dynamic by core id), then
                # broadcast to all 128 partitions
                pid = nc.gpsimd.partition_id()
                off = pid * BS
                nc.gpsimd.dma_start(
                    myrnn_dram.ap(), rnn_nat_dram.ap()[bass.ds(off, BS), :])

                # ---------------- attention (batch-sharded, online softmax) ---------
                with (
                    tc.tile_pool(name="pR", bufs=1) as pR,
                    tc.tile_pool(name="pat", bufs=1) as pat,
                    tc.tile_pool(name="psB", bufs=1, space="PSUM") as psB,
                    tc.tile_pool(name="psS", bufs=1, space="PSUM") as psS,
                ):
                    R = pR.tile([P, BS, H], F32, tag="R")
                    nc.sync.dma_start(
                        R[:],
                        bass.AP(tensor=myrnn_dram, offset=0,
                                ap=[[0, P], [H, BS], [1, H]]))

                    ctx_acc = pat.tile([BS, H], F32, tag="ctx_acc")
                    nc.vector.memset(ctx_acc[:], 0.0)
                    m_run = pat.tile([BS, 1], F32, tag="m_run")
                    nc.vector.memset(m_run[:], -1e30)
                    s_run = pat.tile([BS, 1], F32, tag="s_run")
                    nc.vector.memset(s_run[:], 0.0)
                    scratch = pat.tile([P, S], F32, tag="scratch")
                    diag = pat.tile([P, BS, BS], F32, tag="diag")

                    for c in range(SC):
                        e0 = encp.tile([P, BS, H // 2], F32, tag="enc")
                        nc.sync.dma_start(
                            e0[:], encB.ap()[c * P:(c + 1) * P, :, 0:H // 2])
                        e1 = encp.tile([P, BS, H // 2], F32, tag="enc")
                        nc.sync.dma_start(
                            e1[:], encB.ap()[c * P:(c + 1) * P, :, H // 2:H])

                        acc0 = pat.tile([P, BS], F32, tag="acc0")
                        acc1 = pat.tile([P, BS], F32, tag="acc1")
                        for b in range(BS):
                            nc.vector.scalar_tensor_tensor(
                                out=scratch[:, 0:H // 2], in0=e0[:, b, :],
                                scalar=1.0, in1=R[:, b, 0:H // 2],
                                op0=OP.mult, op1=OP.mult,
                                accum_out=acc0[:, b:b + 1])
                            nc.vector.scalar_tensor_tensor(
                                out=scratch[:, 0:H // 2], in0=e1[:, b, :],
                                scalar=1.0, in1=R[:, b, H // 2:H],
                                op0=OP.mult, op1=OP.mult,
                                accum_out=acc1[:, b:b + 1])
                        sc_t = pat.tile([P, BS], F32, tag="sc_t")
                        nc.vector.tensor_tensor(sc_t[:], acc0[:], acc1[:], OP.add)

                        # chunk max (transpose -> reduce), online max update
                        pt = psT.tile([BS, P], F32, tag="tr16")
                        nc.tensor.transpose(pt[:], sc_t[:], ident[:])
                        scn = pat.tile([BS, P], F32, tag="scn")
                        nc.scalar.copy(scn[:], pt[:])
                        mc = pat.tile([BS, 1], F32, tag="mc")
                        nc.vector.tensor_reduce(mc[:], scn[:], axis=AX.X, op=OP.max)
                        mnew = pat.tile([BS, 1], F32, tag="mnew")
                        nc.vector.tensor_tensor(mnew[:], m_run[:], mc[:], OP.max)
                        fdl = pat.tile([BS, 1], F32, tag="fdl")
                        nc.vector.tensor_tensor(fdl[:], m_run[:], mnew[:],
                                                OP.subtract)
                        fexp = pat.tile([BS, 1], F32, tag="fexp")
                        nc.scalar.activation(fexp[:], fdl[:], AF.Exp)
                        nc.vector.tensor_copy(m_run[:], mnew[:])

                        # broadcast mnew over the 128 s-partitions via DRAM
                        nc.sync.dma_start(mx_dram[c].ap(), mnew[:])
                        mbc = pat.tile([P, BS], F32, tag="mbc")
                        nc.sync.dma_start(
                            mbc[:],
                            bass.AP(tensor=mx_dram[c], offset=0,
                                    ap=[[0, P], [1, BS]]))
                        esb = pat.tile([P, BS], F32, tag="esb")
                        nc.vector.tensor_tensor(esb[:], sc_t[:], mbc[:],
                                                OP.subtract)
                        pT = pat.tile([P, BS], F32, tag="pT")
                        nc.scalar.activation(pT[:], esb[:], AF.Exp)

                        # block-diagonal weights for the context matmul
                        nc.vector.memset(diag[:], 0.0)
                        nc.vector.tensor_copy(
                            bass.AP(tensor=diag.tensor, offset=diag.offset,
                                    ap=[diag.ap[0], [BS + 1, BS]]),
                            pT[:])

                        psc = psB.tile([BS, H], F32, tag="ctx")
                        for b in range(BS):
                            nc.tensor.matmul(psc[:, 0:H // 2], diag[:, b, :],
                                             e0[:, b, :], start=(b == 0),
                                             stop=(b == BS - 1))
                            nc.tensor.matmul(psc[:, H // 2:H], diag[:, b, :],
                                             e1[:, b, :],
                                             start=(b == 0),
                                             stop=(b == BS - 1))
                        pss = psS.tile([BS, 1], F32, tag="sum")
                        nc.tensor.matmul(pss[:], pT[:], ones[:],
                                         start=True, stop=True)

                        nc.vector.scalar_tensor_tensor(
                            out=ctx_acc[:], in0=ctx_acc[:], scalar=fexp[:],
                            in1=psc[:], op0=OP.mult, op1=OP.add)
                        nc.vector.scalar_tensor_tensor(
                            out=s_run[:], in0=s_run[:], scalar=fexp[:],
                            in1=pss[:], op0=OP.mult, op1=OP.add)

                    rinv = pat.tile([BS, 1], F32, tag="rinv")
                    nc.vector.reciprocal(rinv[:], s_run[:])
                    ctxn = pat.tile([BS, H], F32, tag="ctxn")
                    nc.vector.tensor_scalar_mul(out=ctxn[:], in0=ctx_acc[:],
                                                scalar1=rinv[:])
                    bin_, bout = ctx_ag
                    nc.sync.dma_start(bin_.ap(), ctxn[:])
                    nc.gpsimd.collective_compute(
                        "AllGather", OP.bypass, replica_groups=RG,
                        ins=[bin_.ap().opt()], outs=[bout.ap().opt()])
                    ctxf = pat.tile([P, H], F32, tag="ctxf")
                    nc.sync.dma_start(ctxf[:], bout.ap())
                    for j in range(KC):
                        pt = psT.tile([P, P], F32, tag="tr")
                        nc.tensor.transpose(pt[:], ctxf[:, j * P:(j + 1) * P],
                                            ident[:])
                        nc.scalar.copy(ctxT[:, j, :], pt[:])

            # ---------------- concat layer + unembed ---------------------------
            with (
                tc.tile_pool(name="pu", bufs=1) as pu,
                tc.tile_pool(name="puw", bufs=3) as puw,
                tc.tile_pool(name="psU", bufs=2, space="PSUM") as psU,
            ):
                cwt = pu.tile([P, 2 * KC, HS], F32, tag="cwt")
                nc.sync.dma_start(cwt[:], t_layout(cw.ap()))
                ps = psU.tile([HS, B], F32, tag="u")
                for k in range(KC):
                    nc.tensor.matmul(ps[:], cwt[:, k, :], rnnT[:, k, :],
                                     start=(k == 0), stop=False)
                for k in range(KC):
                    nc.tensor.matmul(ps[:], cwt[:, KC + k, :], ctxT[:, k, :],
                                     start=False, stop=(k == KC - 1))
                csl = pu.tile([HS, B], F32, tag="csl")
                nc.scalar.activation(csl[:], ps[:], AF.Tanh, bias=cbt[:])
                bin_, bout = cat_ag
                nc.sync.dma_start(bin_.ap(), csl[:])
                nc.gpsimd.collective_compute(
                    "AllGather", OP.bypass, replica_groups=RG,
                    ins=[bin_.ap().opt()], outs=[bout.ap().opt()])
                nc.sync.dma_start(catT[:], t_layout(bout.ap()))

                obr = pu.tile([P, VS], F32, tag="obr")
                nc.sync.dma_start(
                    obr[:], bass.AP(tensor=ob, offset=0, ap=[[0, P], [1, VS]]))
                exps = pu.tile([B, VS], F32, tag="exps")
                sump = pu.tile([B, VT], F32, tag="sump")
                for t in range(VT):
                    owt = puw.tile([P, KC, VN], F32, tag="owt")
                    nc.sync.dma_start(
                        owt[:], t_layout(owT.ap()[:, t * VN:(t + 1) * VN]))
                    psu = psU.tile([B, VN], F32, tag="u2")
                    for k in range(KC):
                        nc.tensor.matmul(psu[:], catT[:, k, :], owt[:, k, :],
                                         start=(k == 0), stop=(k == KC - 1))
                    lsb = pu.tile([B, VN], F32, tag="lsb")
                    nc.vector.scalar_tensor_tensor(
                        out=lsb[:], in0=psu[:], scalar=1.0,
                        in1=obr[:, t * VN:(t + 1) * VN],
                        op0=OP.mult, op1=OP.add)
                    nc.scalar.activation(exps[:, t * VN:(t + 1) * VN], lsb[:],
                                         AF.Exp, accum_out=sump[:, t:t + 1])
                suml = pu.tile([B, 1], F32, tag="suml")
                nc.vector.tensor_reduce(suml[:], sump[:], axis=AX.X, op=OP.add)
                nc.sync.dma_start(sum_in.ap(), suml[:])
                nc.gpsimd.collective_compute(
                    "AllReduce", OP.add, replica_groups=RG,
                    ins=[sum_in.ap().opt()], outs=[sum_out.ap().opt()])
                ssum = pu.tile([B, 1], F32, tag="ssum")
                nc.sync.dma_start(ssum[:], sum_out.ap())
                sinv = pu.tile([B, 1], F32, tag="sinv")
                nc.vector.reciprocal(sinv[:], ssum[:])
                nc.vector.tensor_scalar_mul(out=exps[:], in0=exps[:],
                                            scalar1=sinv[:])
                nc.sync.dma_start(out_probs.ap(), exps[:])

    nc.compile()
    return nc


_PROG = None


def _sigmoid(x):
    return 1.0 / (1.0 + np.exp(-x))


def _prep_inputs(inputs):
    f = lambda k: np.asarray(inputs[k], dtype=np.float32)
    idx = np.asarray(inputs["input_step"]).astype(np.int64)[0]
    emb = f("emb")
    x0T = np.ascontiguousarray(emb[idx].T)
    h1T = np.ascontiguousarray(f("h1").T)
    c1T = np.ascontiguousarray(f("c1").T)
    mog1_W = f("mog1_W")
    mog1_b = f("mog1_b")
    wih1T = np.ascontiguousarray(f("lstm1_Wih").T)   # [H, 4H]
    whh1T = np.ascontiguousarray(f("lstm1_Whh").T)
    b1 = f("lstm1_bih") + f("lstm1_bhh")             # [4H]
    wih2T = np.ascontiguousarray(f("lstm2_Wih").T)
    b2 = f("lstm2_bih") + f("lstm2_bhh")
    mog2_b = f("mog2_b")
    scale2 = (8.0 * _sigmoid(mog2_b[0]) * _sigmoid(mog2_b[2])
              * _sigmoid(mog2_b[4])).astype(np.float32)          # [H]
    scale2T = np.ascontiguousarray(scale2.reshape(KC, P).T)      # [128, KC]
    fcwT = np.ascontiguousarray(f("fc_W").T)         # [H, H]
    fc_b = f("fc_b")
    cwT = np.ascontiguousarray(f("concat_W").T)      # [2H, H]
    c_b = f("concat_b")
    enc = f("encoder_outputs")
    owT = np.ascontiguousarray(f("out_W").T)         # [H, V]
    o_b = f("out_b")

    in_maps = []
    for i in range(NC):
        sl = slice(P * i, P * (i + 1))
        q4 = [slice(H * q + P * i, H * q + P * (i + 1)) for q in range(4)]
        m = {
            "x0T": x0T, "h1T": h1T, "c1T": c1T,
            "mog1w": np.ascontiguousarray(
                np.stack([mog1_W[r].T[:, sl] for r in range(5)])),
            "mog1b": np.ascontiguousarray(mog1_b[:, sl].T),
            "wih1": np.ascontiguousarray(
                np.concatenate([wih1T[:, q] for q in q4], axis=1)),
            "whh1": np.ascontiguousarray(
                np.concatenate([whh1T[:, q] for q in q4], axis=1)),
            "b1": np.ascontiguousarray(
                np.stack([b1[q] for q in q4], axis=1)),
            "wih2": np.ascontiguousarray(
                np.concatenate([wih2T[:, q4[q]] for q in (0, 2, 3)], axis=1)),
            "b2": np.ascontiguousarray(
                np.stack([b2[q4[q]] for q in (0, 2, 3)], axis=1)),
            "scale2": scale2T,
            "fcw": np.ascontiguousarray(fcwT[:, sl]),
            "fcb": np.ascontiguousarray(fc_b[sl][:, None]),
            "cw": np.ascontiguousarray(cwT[:, sl]),
            "cb": np.ascontiguousarray(c_b[sl][:, None]),
            "encB": np.ascontiguousarray(enc[:, BS * i:BS * (i + 1), :]),
            "owT": np.ascontiguousarray(owT[:, VS * i:VS * (i + 1)]),
            "ob": np.ascontiguousarray(o_b[VS * i:VS * (i + 1)][None, :]),
        }
        in_maps.append(m)
    return in_maps


def run(inputs, trace=False):
    global _PROG
    if _PROG is None:
        _PROG = build_program()
    in_maps = _prep_inputs(inputs)
    res = run_bass_kernel_spmd(_PROG, in_maps, core_ids=list(range(NC)),
                               trace=trace)
    probs = np.concatenate([res.results[i]["out_probs"] for i in range(NC)],
                           axis=1)
    hidden = np.ascontiguousarray(res.results[0]["h2T_out"].T)[None]
    return (probs, hidden), res


def kernel(**inputs):
    out, _ = run(inputs)
    return out


# revision 20
# speedup vs baseline: 2.1525x; 2.1525x over previous
"""Trainium2 Bass kernel for nn_LuongAttnDecoderMogLSTM.

Strategy (8 NeuronCores on one chip):
  - MogLSTM / fc / concat matmuls: weight-sharded over the output dim
    (1/8 slice per core), activations kept transposed [feature, batch];
    slices AllGathered after each step (gather = concat on first dim).
    The mogrifier's 2x gate factors are folded into host-scaled weights
    (exact powers of two), so each gather applies with a single in-place
    multiply-accumulate DMA.
  - Layer-2 MogLSTM has zero-init state, so its mogrifier collapses to a
    constant per-feature scale folded into lstm2_Wih; lstm2_Whh and the
    f-gate are never needed.
  - Luong attention: batch-sharded (16 rows per core), single pass over
    encoder_outputs; per-s-chunk independent softmax normalization with a
    deferred max/renormalize combine (chunks fully pipeline). Scores on
    DVE (fused multiply+accumulate, fp32); context via block-diagonal PE
    matmuls in bf16 (fp32 matmul runs at 1/4 rate).
  - Output projection: vocab-sharded (4000 cols per core) in bf16; softmax
    uses one AllReduce of per-core exp-sums (logits are tanh-bounded, so
    no max subtraction is needed).
"""
import sys

for _p in ("/opt/trn_rl_repo", "/root/.axon_site/_ro/trn_rl_repo"):
    if _p not in sys.path:
        sys.path.insert(0, _p)

import numpy as np
import ml_dtypes

import concourse.bass as bass
import concourse.mybir as mybir
import concourse.tile as tile
from concourse import bacc
from concourse.bass_utils import run_bass_kernel_spmd
from concourse.masks import make_identity

F32 = mybir.dt.float32
BF16 = mybir.dt.bfloat16
AF = mybir.ActivationFunctionType
OP = mybir.AluOpType
AX = mybir.AxisListType

NC = 8          # cores
B = 128         # batch
H = 1024        # hidden
S = 512         # encoder length
V = 32000       # vocab
P = 128         # partitions
KC = H // P     # 8 k-chunks of 128
HS = H // NC    # 128   per-core slice of H-sized outputs
GQ = 4          # lstm1 quadrants (i,f,g,o)
G2Q = 3         # lstm2 quadrants kept (i,g,o)
BS = B // NC    # 16    batch shard for attention
SC = 4          # s-chunks of 128
VS = V // NC    # 4000  vocab slice
VT = 8          # vocab n-chunks of 500
VN = VS // VT   # 500
HH = H // 2     # 512

RG = [list(range(NC))]


def _ag_pair(nc, name, in_shape, dtype=F32):
    bin_ = nc.dram_tensor(f"{name}_agin", list(in_shape), dtype)
    out_shape = [in_shape[0] * NC] + list(in_shape[1:])
    bout = nc.dram_tensor(f"{name}_agout", out_shape, dtype, addr_space="Shared")
    return bin_, bout


def build_program():
    nc = bacc.Bacc("TRN2", target_bir_lowering=False, debug=False, num_devices=NC)

    def inp(name, shape, dtype=F32):
        return nc.dram_tensor(name, list(shape), dtype, kind="ExternalInput")

    x0T = inp("x0T", [H, B])
    h1T = inp("h1T", [H, B])
    c1T = inp("c1T", [H, B])
    mog1w = inp("mog1w", [5, H, HS])      # (scaled) W_r.T column slice
    mog1b = inp("mog1b", [P, 5])
    wih1 = inp("wih1", [H, GQ * P])       # quadrant-interleaved col slice, x8
    whh1 = inp("whh1", [H, GQ * P])       # x4
    b1 = inp("b1", [P, GQ])
    wih2 = inp("wih2", [H, G2Q * P])      # scale2-folded col slice
    b2 = inp("b2", [P, G2Q])
    fcw = inp("fcw", [H, HS])
    fcb = inp("fcb", [P, 1])
    cw = inp("cw", [2 * H, HS])
    cb = inp("cb", [P, 1])
    encB = inp("encB", [S, BS, H])
    owT = inp("owT", [H, VS], BF16)
    ob = inp("ob", [1, VS])

    out_probs = nc.dram_tensor("out_probs", [B, VS], F32, kind="ExternalOutput")
    h2T_out = nc.dram_tensor("h2T_out", [H, B], F32, kind="ExternalOutput")

    # collective bounce buffers
    mog_ag = [_ag_pair(nc, f"mog{r}", [HS, B]) for r in range(5)]
    g1_ag = _ag_pair(nc, "g1", [GQ * P, B])
    g2_ag = _ag_pair(nc, "g2", [G2Q * P, B])
    fc_ag = _ag_pair(nc, "fc", [HS, B])
    ctx_ag = _ag_pair(nc, "ctx", [BS, H])
    cat_ag = _ag_pair(nc, "cat", [HS, B])
    sum_in = nc.dram_tensor("sum_agin", [B, 1], F32)
    sum_out = nc.dram_tensor("sum_agout", [B, 1], F32, addr_space="Shared")

    rnn_nat_dram = nc.dram_tensor("rnn_nat_dram", [B, H], F32)
    myrnn_dram = nc.dram_tensor("myrnn_dram", [BS, H], F32)
    mx_dram = [nc.dram_tensor(f"mx_dram{c}", [BS, 1], F32) for c in range(SC)]

    def t_layout(ap):  # [H, X] dram -> [128, KC, X] (h = 128*chunk + p)
        return ap.rearrange("(o p) b -> p o b", p=P)

    with tile.TileContext(nc) as tc:
        with (
            tc.tile_pool(name="pp", bufs=1) as pp,
            tc.tile_pool(name="psT", bufs=1, space="PSUM") as psT,
        ):
            ident = pp.tile([P, P], F32, tag="ident")
            make_identity(nc, ident[:])
            ones = pp.tile([P, 1], F32, tag="ones")
            nc.vector.memset(ones[:], 1.0)

            # persistent activations (cross-phase)
            rnnT = pp.tile([P, KC, B], F32, tag="rnnT")

            # small constants
            m1b = pp.tile([P, 5], F32, tag="m1b")
            nc.sync.dma_start(m1b[:], mog1b.ap())
            b1t = pp.tile([P, GQ], F32, tag="b1t")
            nc.sync.dma_start(b1t[:], b1.ap())
            b2t = pp.tile([P, G2Q], F32, tag="b2t")
            nc.sync.dma_start(b2t[:], b2.ap())
            fcbt = pp.tile([P, 1], F32, tag="fcbt")
            nc.sync.dma_start(fcbt[:], fcb.ap())
            cbt = pp.tile([P, 1], F32, tag="cbt")
            nc.sync.dma_start(cbt[:], cb.ap())

            with tc.tile_pool(name="encp", bufs=2) as encp:
                # ---------------- phase A: MogLSTM + fc (weight-sharded) -------------
                with (
                    tc.tile_pool(name="pa", bufs=1) as pa,
                    tc.tile_pool(name="paw", bufs=3) as paw,
                    tc.tile_pool(name="psA", bufs=2, space="PSUM") as psA,
                ):
                    xt = pa.tile([P, KC, B], F32, tag="xt")
                    nc.sync.dma_start(xt[:], t_layout(x0T.ap()))
                    ht = pa.tile([P, KC, B], F32, tag="ht")
                    nc.sync.dma_start(ht[:], t_layout(h1T.ap()))
                    ct = pa.tile([P, KC, B], F32, tag="ct")
                    nc.sync.dma_start(ct[:], t_layout(c1T.ap()))

                    # 5 mogrifier rounds (2x factors folded into weights; the
                    # gathered sigmoid slice applies via in-place multiply DMA)
                    for r in range(5):
                        wt = paw.tile([P, KC, HS], F32, tag="mogw")
                        nc.sync.dma_start(wt[:], t_layout(mog1w.ap()[r]))
                        src = ht if r % 2 == 0 else xt
                        tgt = xt if r % 2 == 0 else ht
                        ps = psA.tile([HS, B], F32, tag="g")
                        for k in range(KC):
                            nc.tensor.matmul(ps[:], wt[:, k, :], src[:, k, :],
                                             start=(k == 0), stop=(k == KC - 1))
                        gs = pa.tile([HS, B], F32, tag="gs")
                        nc.scalar.activation(gs[:], ps[:], AF.Sigmoid,
                                             bias=m1b[:, r:r + 1])
                        bin_, bout = mog_ag[r]
                        nc.sync.dma_start(bin_.ap(), gs[:])
                        nc.gpsimd.collective_compute(
                            "AllGather", OP.bypass, replica_groups=RG,
                            ins=[bin_.ap().opt()], outs=[bout.ap().opt()])
                        gfull = pa.tile([P, KC, B], F32, tag="gfull")
                        nc.sync.dma_start(gfull[:], t_layout(bout.ap()))
                        nc.vector.tensor_tensor(tgt[:], gfull[:], tgt[:],
                                                OP.mult)

                    # lstm1 gates (4 quadrant chunks, sliced by core)
                    gsl = pa.tile([P, GQ, B], F32, tag="gsl")
                    for q in range(GQ):
                        wi = paw.tile([P, KC, P], F32, tag="wq")
                        nc.sync.dma_start(
                            wi[:], t_layout(wih1.ap()[:, q * P:(q + 1) * P]))
                        wh = paw.tile([P, KC, P], F32, tag="wq")
                        nc.sync.dma_start(
                            wh[:], t_layout(whh1.ap()[:, q * P:(q + 1) * P]))
                        ps = psA.tile([P, B], F32, tag="g")
                        for k in range(KC):
                            nc.tensor.matmul(ps[:], wi[:, k, :], xt[:, k, :],
                                             start=(k == 0), stop=False)
                        for k in range(KC):
                            nc.tensor.matmul(ps[:], wh[:, k, :], ht[:, k, :],
                                             start=False, stop=(k == KC - 1))
                        fn = AF.Tanh if q == 2 else AF.Sigmoid
                        nc.scalar.activation(gsl[:, q, :], ps[:], fn,
                                             bias=b1t[:, q:q + 1])
                    bin_, bout = g1_ag
                    nc.sync.dma_start(
                        bin_.ap().rearrange("(q p) b -> p q b", p=P), gsl[:])
                    nc.gpsimd.collective_compute(
                        "AllGather", OP.bypass, replica_groups=RG,
                        ins=[bin_.ap().opt()], outs=[bout.ap().opt()])
                    g1v = bout.ap().rearrange("(r q p) b -> q p r b", q=GQ, p=P)
                    quad = []
                    for q in range(GQ):
                        qt = pa.tile([P, KC, B], F32, tag=f"quad{q}")
                        nc.sync.dma_start(qt[:], g1v[q])
                        quad.append(qt)
                    si, sf, tg, so = quad

                    # cell 1 (transposed layout); temp tags chained to bound SBUF
                    t1 = pa.tile([P, KC, B], F32, tag="tmpA")
                    nc.vector.tensor_tensor(t1[:], si[:], tg[:], OP.mult)
                    t2 = pa.tile([P, KC, B], F32, tag="tmpB")
                    nc.vector.tensor_tensor(t2[:], sf[:], ct[:], OP.mult)
                    c1n = pa.tile([P, KC, B], F32, tag="tmpC")
                    nc.vector.tensor_tensor(c1n[:], t1[:], t2[:], OP.add)
                    tc1 = pa.tile([P, KC, B], F32, tag="tmpA")
                    nc.scalar.activation(tc1[:], c1n[:], AF.Tanh)
                    h1n = pa.tile([P, KC, B], F32, tag="tmpB")
                    nc.vector.tensor_tensor(h1n[:], so[:], tc1[:], OP.mult)

                    # layer 2 gates directly on h1n (scale2 folded into wih2)
                    g2sl = pa.tile([P, G2Q, B], F32, tag="g2sl")
                    for q in range(G2Q):
                        wi = paw.tile([P, KC, P], F32, tag="wq")
                        nc.sync.dma_start(
                            wi[:], t_layout(wih2.ap()[:, q * P:(q + 1) * P]))
                        ps = psA.tile([P, B], F32, tag="g")
                        for k in range(KC):
                            nc.tensor.matmul(ps[:], wi[:, k, :], h1n[:, k, :],
                                             start=(k == 0), stop=(k == KC - 1))
                        fn = AF.Tanh if q == 1 else AF.Sigmoid
                        nc.scalar.activation(g2sl[:, q, :], ps[:], fn,
                                             bias=b2t[:, q:q + 1])
                    bin_, bout = g2_ag
                    nc.sync.dma_start(
                        bin_.ap().rearrange("(q p) b -> p q b", p=P), g2sl[:])
                    nc.gpsimd.collective_compute(
                        "AllGather", OP.bypass, replica_groups=RG,
                        ins=[bin_.ap().opt()], outs=[bout.ap().opt()])
                    g2v = bout.ap().rearrange("(r q p) b -> q p r b", q=G2Q, p=P)
                    q2 = []
                    for q in range(G2Q):
                        qt = pa.tile([P, KC, B], F32, tag=f"quad{q}")
                        nc.sync.dma_start(qt[:], g2v[q])
                        q2.append(qt)
                    si2, tg2, so2 = q2
                    c2n = pa.tile([P, KC, B], F32, tag="tmpA")
                    nc.vector.tensor_tensor(c2n[:], si2[:], tg2[:], OP.mult)
                    tc2 = pa.tile([P, KC, B], F32, tag="tmpC")
                    nc.scalar.activation(tc2[:], c2n[:], AF.Tanh)
                    h2 = pa.tile([P, KC, B], F32, tag="h2")
                    nc.vector.tensor_tensor(h2[:], so2[:], tc2[:], OP.mult)
                    nc.sync.dma_start(t_layout(h2T_out.ap()), h2[:])

                    # fc -> rnn_out (transposed, gathered)
                    fw = paw.tile([P, KC, HS], F32, tag="mogw")
                    nc.sync.dma_start(fw[:], t_layout(fcw.ap()))
                    ps = psA.tile([HS, B], F32, tag="g")
                    for k in range(KC):
                        nc.tensor.matmul(ps[:], fw[:, k, :], h2[:, k, :],
                                         start=(k == 0), stop=(k == KC - 1))
                    rsl = pa.tile([HS, B], F32, tag="rsl")
                    nc.scalar.activation(rsl[:], ps[:], AF.Identity, bias=fcbt[:])
                    bin_, bout = fc_ag
                    nc.sync.dma_start(bin_.ap(), rsl[:])
                    nc.gpsimd.collective_compute(
                        "AllGather", OP.bypass, replica_groups=RG,
                        ins=[bin_.ap().opt()], outs=[bout.ap().opt()])
                    nc.sync.dma_start(rnnT[:], t_layout(bout.ap()))

                    # rnn_out natural [B, H] -> dram (for the per-core slice bcast)
                    rnat = pa.tile([P, H], F32, tag="rnat")
                    for j in range(KC):
                        pt = psT.tile([P, P], F32, tag="tr")
                        nc.tensor.transpose(pt[:], rnnT[:, j, :], ident[:])
                        nc.scalar.copy(rnat[:, j * P:(j + 1) * P], pt[:])
                    nc.sync.dma_start(rnn_nat_dram.ap(), rnat[:])

                # my 16 batch rows of rnn_out (dynamic by core id), then
                # broadcast to all 128 partitions
                pid = nc.gpsimd.partition_id()
                off = pid * BS
                nc.gpsimd.dma_start(
                    myrnn_dram.ap(), rnn_nat_dram.ap()[bass.ds(off, BS), :])

                # ---------------- attention (batch-sharded, chunkwise softmax) ------
                with (
                    tc.tile_pool(name="pR", bufs=1) as pR,
                    tc.tile_pool(name="pat", bufs=1) as pat,
                    tc.tile_pool(name="pc2", bufs=2) as pc2,
                    tc.tile_pool(name="pcx", bufs=SC) as pcx,
                    tc.tile_pool(name="peb", bufs=2) as peb,
                    tc.tile_pool(name="psB", bufs=2, space="PSUM") as psB,
                    tc.tile_pool(name="psS", bufs=2, space="PSUM") as psS,
                ):
                    R = pR.tile([P, BS, H], F32, tag="R")
                    nc.sync.dma_start(
                        R[:],
                        bass.AP(tensor=myrnn_dram, offset=0,
                                ap=[[0, P], [H, BS], [1, H]]))
                    scratch = pat.tile([P, HH], F32, tag="scratch")

                    ctx_cs, sum_cs, mc_cs = [], [], []
                    for c in range(SC):
                        e0 = encp.tile([P, BS, HH], F32, tag="enc")
                        nc.sync.dma_start(
                            e0[:], encB.ap()[c * P:(c + 1) * P, :, 0:HH])
                        e1 = encp.tile([P, BS, HH], F32, tag="enc")
                        nc.sync.dma_start(
                            e1[:], encB.ap()[c * P:(c + 1) * P, :, HH:H])
                        # bf16 copies for the context matmul (ACT)
                        e0b = peb.tile([P, BS, HH], BF16, tag="encb")
                        nc.scalar.copy(e0b[:], e0[:])
                        e1b = peb.tile([P, BS, HH], BF16, tag="encb")
                        nc.scalar.copy(e1b[:], e1[:])

                        acc0 = pc2.tile([P, BS], F32, tag="acc0")
                        acc1 = pc2.tile([P, BS], F32, tag="acc1")
                        for b in range(BS):
                            nc.vector.scalar_tensor_tensor(
                                out=scratch[:], in0=e0[:, b, :],
                                scalar=1.0, in1=R[:, b, 0:HH],
                                op0=OP.mult, op1=OP.mult,
                                accum_out=acc0[:, b:b + 1])
                            nc.vector.scalar_tensor_tensor(
                                out=scratch[:], in0=e1[:, b, :],
                                scalar=1.0, in1=R[:, b, HH:H],
                                op0=OP.mult, op1=OP.mult,
                                accum_out=acc1[:, b:b + 1])
                        sc_t = pc2.tile([P, BS], F32, tag="sc_t")
                        nc.vector.tensor_tensor(sc_t[:], acc0[:], acc1[:], OP.add)

                        # chunk max (transpose -> reduce) and partition bcast
                        pt = psT.tile([BS, P], F32, tag="tr16")
                        nc.tensor.transpose(pt[:], sc_t[:], ident[:])
                        scn = pc2.tile([BS, P], F32, tag="scn")
                        nc.scalar.copy(scn[:], pt[:])
                        mc = pcx.tile([BS, 1], F32, tag="mc")
                        nc.vector.tensor_reduce(mc[:], scn[:], axis=AX.X, op=OP.max)
                        mc_cs.append(mc)
                        nc.sync.dma_start(mx_dram[c].ap(), mc[:])
                        mbc = pc2.tile([P, BS], F32, tag="mbc")
                        nc.sync.dma_start(
                            mbc[:],
                            bass.AP(tensor=mx_dram[c], offset=0,
                                    ap=[[0, P], [1, BS]]))
                        esb = pc2.tile([P, BS], F32, tag="esb")
                        nc.vector.tensor_tensor(esb[:], sc_t[:], mbc[:],
                                                OP.subtract)
                        pT = pc2.tile([P, BS], F32, tag="pT")
                        nc.scalar.activation(pT[:], esb[:], AF.Exp)

                        # block-diagonal bf16 weights for the context matmul
                        diag = pc2.tile([P, BS, BS], BF16, tag="diag")
                        nc.vector.memset(diag[:], 0.0)
                        nc.vector.tensor_copy(
                            bass.AP(tensor=diag.tensor, offset=diag.offset,
                                    ap=[diag.ap[0], [BS + 1, BS]]),
                            pT[:])

                        psc = psB.tile([BS, H], F32, tag="ctx")
                        for b in range(BS):
                            nc.tensor.matmul(psc[:, 0:HH], diag[:, b, :],
                                             e0b[:, b, :], start=(b == 0),
                                             stop=(b == BS - 1))
                            nc.tensor.matmul(psc[:, HH:H], diag[:, b, :],
                                             e1b[:, b, :],
                                             start=(b == 0),
                                             stop=(b == BS - 1))
                        pss = psS.tile([BS, 1], F32, tag="sum")
                        nc.tensor.matmul(pss[:], pT[:], ones[:],
                                         start=True, stop=True)
                        ctx_c = pcx.tile([BS, H], BF16, tag="ctx_c")
                        nc.scalar.copy(ctx_c[:], psc[:])
                        sum_c = pcx.tile([BS, 1], F32, tag="sum_c")
                        nc.scalar.copy(sum_c[:], pss[:])
                        ctx_cs.append(ctx_c)
                        sum_cs.append(sum_c)

                    # combine chunks: global max, weights, weighted sums
                    m01 = pat.tile([BS, 1], F32, tag="m01")
                    nc.vector.tensor_tensor(m01[:], mc_cs[0][:], mc_cs[1][:], OP.max)
                    m23 = pat.tile([BS, 1], F32, tag="m23")
                    nc.vector.tensor_tensor(m23[:], mc_cs[2][:], mc_cs[3][:], OP.max)
                    mg = pat.tile([BS, 1], F32, tag="mg")
                    nc.vector.tensor_tensor(mg[:], m01[:], m23[:], OP.max)
                    ws = []
                    for c in range(SC):
                        wd = pc2.tile([BS, 1], F32, tag="wd")
                        nc.vector.tensor_tensor(wd[:], mc_cs[c][:], mg[:],
                                                OP.subtract)
                        w = pcx.tile([BS, 1], F32, tag="wexp")
                        nc.scalar.activation(w[:], wd[:], AF.Exp)
                        ws.append(w)
                    ctx_acc = pat.tile([BS, H], F32, tag="ctx_acc")
                    nc.vector.tensor_scalar_mul(out=ctx_acc[:], in0=ctx_cs[0][:],
                                                scalar1=ws[0][:])
                    for c in range(1, SC):
                        nc.vector.scalar_tensor_tensor(
                            out=ctx_acc[:], in0=ctx_cs[c][:], scalar=ws[c][:],
                            in1=ctx_acc[:], op0=OP.mult, op1=OP.add)
                    s_tot = pat.tile([BS, 1], F32, tag="s_tot")
                    nc.vector.tensor_scalar_mul(out=s_tot[:], in0=sum_cs[0][:],
                                                scalar1=ws[0][:])
                    for c in range(1, SC):
                        nc.vector.scalar_tensor_tensor(
                            out=s_tot[:], in0=sum_cs[c][:], scalar=ws[c][:],
                            in1=s_tot[:], op0=OP.mult, op1=OP.add)
                    rinv = pat.tile([BS, 1], F32, tag="rinv")
                    nc.vector.reciprocal(rinv[:], s_tot[:])
                    nc.vector.tensor_scalar_mul(out=ctx_acc[:], in0=ctx_acc[:],
                                                scalar1=rinv[:])
                    bin_, bout = ctx_ag
                    nc.sync.dma_start(bin_.ap(), ctx_acc[:])
                    nc.gpsimd.collective_compute(
                        "AllGather", OP.bypass, replica_groups=RG,
                        ins=[bin_.ap().opt()], outs=[bout.ap().opt()])
                    ctx_bout = bout

            # ---------------- concat layer + unembed ---------------------------
            with (
                tc.tile_pool(name="pu", bufs=1) as pu,
                tc.tile_pool(name="puw", bufs=3) as puw,
                tc.tile_pool(name="psU", bufs=2, space="PSUM") as psU,
            ):
                # gathered context -> transposed layout
                ctxT = pu.tile([P, KC, B], F32, tag="ctxT")
                ctxf = pu.tile([P, H], F32, tag="ctxf")
                nc.sync.dma_start(ctxf[:], ctx_bout.ap())
                for j in range(KC):
                    pt = psT.tile([P, P], F32, tag="tr")
                    nc.tensor.transpose(pt[:], ctxf[:, j * P:(j + 1) * P],
                                        ident[:])
                    nc.scalar.copy(ctxT[:, j, :], pt[:])

                cwt = pu.tile([P, 2 * KC, HS], F32, tag="cwt")
                nc.sync.dma_start(cwt[:], t_layout(cw.ap()))
                ps = psU.tile([HS, B], F32, tag="u")
                for k in range(KC):
                    nc.tensor.matmul(ps[:], cwt[:, k, :], rnnT[:, k, :],
                                     start=(k == 0), stop=False)
                for k in range(KC):
                    nc.tensor.matmul(ps[:], cwt[:, KC + k, :], ctxT[:, k, :],
                                     start=False, stop=(k == KC - 1))
                csl = pu.tile([HS, B], F32, tag="csl")
                nc.scalar.activation(csl[:], ps[:], AF.Tanh, bias=cbt[:])
                bin_, bout = cat_ag
                nc.sync.dma_start(bin_.ap(), csl[:])
                nc.gpsimd.collective_compute(
                    "AllGather", OP.bypass, replica_groups=RG,
                    ins=[bin_.ap().opt()], outs=[bout.ap().opt()])
                catT = pu.tile([P, KC, B], F32, tag="catT")
                nc.sync.dma_start(catT[:], t_layout(bout.ap()))
                cb16 = pu.tile([P, KC, B], BF16, tag="cb16")
                nc.scalar.copy(cb16[:], catT[:])

                obr = pu.tile([P, VS], F32, tag="obr")
                nc.sync.dma_start(
                    obr[:], bass.AP(tensor=ob, offset=0, ap=[[0, P], [1, VS]]))
                exps = pu.tile([B, VS], F32, tag="exps")
                sump = pu.tile([B, VT], F32, tag="sump")
                for t in range(VT):
                    owt = puw.tile([P, KC, VN], BF16, tag="owt")
                    nc.sync.dma_start(
                        owt[:], t_layout(owT.ap()[:, t * VN:(t + 1) * VN]))
                    psu = psU.tile([B, VN], F32, tag="u2")
                    for k in range(KC):
                        nc.tensor.matmul(psu[:], cb16[:, k, :], owt[:, k, :],
                                         start=(k == 0), stop=(k == KC - 1))
                    lsb = pu.tile([B, VN], F32, tag="lsb")
                    nc.vector.scalar_tensor_tensor(
                        out=lsb[:], in0=psu[:], scalar=1.0,
                        in1=obr[:, t * VN:(t + 1) * VN],
                        op0=OP.mult, op1=OP.add)
                    nc.scalar.activation(exps[:, t * VN:(t + 1) * VN], lsb[:],
                                         AF.Exp, accum_out=sump[:, t:t + 1])
                suml = pu.tile([B, 1], F32, tag="suml")
                nc.vector.tensor_reduce(suml[:], sump[:], axis=AX.X, op=OP.add)
                nc.sync.dma_start(sum_in.ap(), suml[:])
                nc.gpsimd.collective_compute(
                    "AllReduce", OP.add, replica_groups=RG,
                    ins=[sum_in.ap().opt()], outs=[sum_out.ap().opt()])
                ssum = pu.tile([B, 1], F32, tag="ssum")
                nc.sync.dma_start(ssum[:], sum_out.ap())
                sinv = pu.tile([B, 1], F32, tag="sinv")
                nc.vector.reciprocal(sinv[:], ssum[:])
                nc.vector.tensor_scalar_mul(out=exps[:], in0=exps[:],
                                            scalar1=sinv[:])
                nc.sync.dma_start(out_probs.ap(), exps[:])

    nc.compile()
    return nc


_PROG = None


def _sigmoid(x):
    return 1.0 / (1.0 + np.exp(-x))


def _prep_inputs(inputs):
    f = lambda k: np.asarray(inputs[k], dtype=np.float32)
    idx = np.asarray(inputs["input_step"]).astype(np.int64)[0]
    emb = f("emb")
    x0T = np.ascontiguousarray(emb[idx].T)
    h1T = np.ascontiguousarray(f("h1").T)
    c1T = np.ascontiguousarray(f("c1").T)
    # fold the mogrifier 2x gate factors into downstream weights (exact)
    mog1_W = f("mog1_W").copy()
    mog1_W[1] *= 2.0
    mog1_W[2] *= 2.0
    mog1_W[3] *= 4.0
    mog1_W[4] *= 4.0
    mog1_b = f("mog1_b")
    wih1T = np.ascontiguousarray(f("lstm1_Wih").T) * 8.0   # [H, 4H], x-scale
    whh1T = np.ascontiguousarray(f("lstm1_Whh").T) * 4.0   # h-scale
    b1 = f("lstm1_bih") + f("lstm1_bhh")                   # [4H]
    mog2_b = f("mog2_b")
    scale2 = (8.0 * _sigmoid(mog2_b[0]) * _sigmoid(mog2_b[2])
              * _sigmoid(mog2_b[4])).astype(np.float32)    # [H]
    wih2T = np.ascontiguousarray(f("lstm2_Wih").T) * scale2[:, None]
    b2 = f("lstm2_bih") + f("lstm2_bhh")
    fcwT = np.ascontiguousarray(f("fc_W").T)
    fc_b = f("fc_b")
    cwT = np.ascontiguousarray(f("concat_W").T)
    c_b = f("concat_b")
    enc = f("encoder_outputs")
    owT = np.ascontiguousarray(f("out_W").T).astype(ml_dtypes.bfloat16)
    o_b = f("out_b")

    in_maps = []
    for i in range(NC):
        sl = slice(P * i, P * (i + 1))
        q4 = [slice(H * q + P * i, H * q + P * (i + 1)) for q in range(4)]
        m = {
            "x0T": x0T, "h1T": h1T, "c1T": c1T,
            "mog1w": np.ascontiguousarray(
                np.stack([mog1_W[r].T[:, sl] for r in range(5)])),
            "mog1b": np.ascontiguousarray(mog1_b[:, sl].T),
            "wih1": np.ascontiguousarray(
                np.concatenate([wih1T[:, q] for q in q4], axis=1)),
            "whh1": np.ascontiguousarray(
                np.concatenate([whh1T[:, q] for q in q4], axis=1)),
            "b1": np.ascontiguousarray(
                np.stack([b1[q] for q in q4], axis=1)),
            "wih2": np.ascontiguousarray(
                np.concatenate([wih2T[:, q4[q]] for q in (0, 2, 3)], axis=1)),
            "b2": np.ascontiguousarray(
                np.stack([b2[q4[q]] for q in (0, 2, 3)], axis=1)),
            "fcw": np.ascontiguousarray(fcwT[:, sl]),
            "fcb": np.ascontiguousarray(fc_b[sl][:, None]),
            "cw": np.ascontiguousarray(cwT[:, sl]),
            "cb": np.ascontiguousarray(c_b[sl][:, None]),
            "encB": np.ascontiguousarray(enc[:, BS * i:BS * (i + 1), :]),
            "owT": np.ascontiguousarray(owT[:, VS * i:VS * (i + 1)]),
            "ob": np.ascontiguousarray(o_b[VS * i:VS * (i + 1)][None, :]),
        }
        in_maps.append(m)
    return in_maps


def run(inputs, trace=False):
    global _PROG
    if _PROG is None:
        _PROG = build_program()
    in_maps = _prep_inputs(inputs)
    res = run_bass_kernel_spmd(_PROG, in_maps, core_ids=list(range(NC)),
                               trace=trace)
    probs = np.concatenate([res.results[i]["out_probs"] for i in range(NC)],
                           axis=1)
    hidden = np.ascontiguousarray(res.results[0]["h2T_out"].T)[None]
    return (probs, hidden), res


def kernel(**inputs):
    out, _ = run(inputs)
    return out
